# revision 37
# baseline (speedup 1.0000x reference)
"""AUC pairwise loss kernel for Trainium2, SPMD over 8 NeuronCores. v2.

Reference computation (N = 16384):
    pred = softmax(y_pred)[:, 1]                       # (N,)
    a_i  = pred_i + GAMMA   for rows with y_true == 1  ("neg" axis)
    b_j  = pred_j           for rows with y_true == 0  ("pos" axis)
    S2   = sum_{a_i > b_j} (a_i - b_j)^2,  C = #{a_i > b_j}
    auc  = S2 / max(C, 1)
    bce  = -mean(yt*clip(log pred, -100) + (1-yt)*clip(log(1-pred), -100))
    loss = ALPHA*bce + (1.0-ALPHA)*auc   (bce alone if C == 0)

Sharding: both classes are sorted by logit z1-z0 (monotone in pred).
"a" rows (yt==1) are dealt round-robin by sorted rank across 8 cores
(~1037 each, padded to A=1152); every core gets all "b" rows (yt==0,
padded to B=8192).  BCE is row-sharded N/8 per core.

With both sides sorted, ind[q, i] = (a_i > b_q) is a monotone
staircase.  For each 128-lane b block c the host finds the exact
uncertain a-range [lo_c, hi_c) from f64 sigmoid bounds over global
rank windows (margin EPS); measured widths are <= 31, so each block
gets a uniform W=32-col band at off_c = min(lo_c, A-W).  Columns left
of the band are certainly all-0 (skipped), columns right of
s_c = off_c + W are certainly all-1 (handled by a "step" matmul).

Device per-rep work:
  DVE: 2x tensor_tensor is_gt over the band layout [128, 32*W] f32
       (a broadcast along partitions, b -{} gamma expanded per block)
       -> bf16 indicator.
  PE:  3 step matmuls (stationary = hi/lo bf16 split of per-block
       weight column-sums [64, 14], moving = host 0/1 suffix matrix
       [64, bank]) that also zero psum rows 0:14 via start=True, then
       64 band matmuls (stationary = block weights [128, 7] bf16 =
       [w0 | -2b hi/mid/lo | b^2 hi/mid/lo], moving = indicator band
       [128, 32]) accumulating into psum[0:7, off_c:off_c+W].
       LDWEIGHTS cost ~ columns (7 or 14), so weight loads are ~6 ns.
Epilogue: DMA psum [14, A] and the BCE row-sums straight to DRAM; the
host applies the masked a-polynomials in f64:
  S2 = sum_i m_i (a_i^2 K_i + a_i S1_i + S2c_i),  C = sum_i m_i K_i
with K = rows 0+7, S1 = rows 1:4 + 8:11 (pre-scaled by -2),
S2c = rows 4:7 + 11:14.
"""

import numpy as np

from concourse import bacc, bass, mybir, tile
from concourse.bass_utils import run_bass_kernel_spmd

N = 16384
NCORES = 8
P = 128
GAMMA = 0.15
ALPHA = 0.6
A_PAD = 2.5                  # pad "a" slots sort last; masked out on host
EPS = 1e-4                   # host-vs-device sigmoid classification margin

F32 = mybir.dt.float32
BF16 = mybir.dt.bfloat16
AF = mybir.ActivationFunctionType
OP = mybir.AluOpType
NW = 7  # weight cols: w0, bh, bm, bl, b2h, b2m, b2l


def build_nc(A, B, classes, off, wid, real_end=None, debug=False, repeat=1):
    """A: per-core padded 'a' count; B: padded 'b' count.
    classes: list of (Wg, [block ids]) width classes; off/wid: per-block
    band start and exact width (wid == 0 -> no matmul for the block).
    The band buffer concatenates the classes; block k of class g owns
    cols [base_g + k*Wg, base_g + (k+1)*Wg) of which the first wid[c]
    are streamed into PE.  Band matmuls alternate between PE col-groups
    0 and 32 (concurrent sub-arrays); col-group 32's psum rows are
    zeroed once before the loop, so only group 0 carries the per-rep
    step matmuls.  real_end: # of non-pad 'a' cols (psum cols beyond it
    stay garbage and the host ignores them).  repeat>1 re-runs the main
    loop for slope benchmarking."""
    NBLK = B // P
    assert len(off) == NBLK and len(wid) == NBLK
    if real_end is None:
        real_end = A
    G2 = 32  # partition offset of the second matmul col-group
    cbase = []
    BAND = 0
    for Wg, blks in classes:
        cbase.append(BAND)
        BAND += Wg * len(blks)
    # (class, slot, col base) per block with a band
    slot = {}
    for g, (Wg, blks) in enumerate(classes):
        for k, c in enumerate(blks):
            slot[c] = (g, cbase[g] + k * Wg)

    nc = bacc.Bacc("TRN2", target_bir_lowering=False, debug=debug)

    z_band = nc.dram_tensor("z_band", [BAND], F32, kind="ExternalInput")
    yp_b = nc.dram_tensor("yp_b", [B, 2], F32, kind="ExternalInput")
    mb = nc.dram_tensor("mb", [B], F32, kind="ExternalInput")   # 1=pad
    s_mat = nc.dram_tensor("s_mat", [NBLK, A], BF16, kind="ExternalInput")
    yp_s = nc.dram_tensor("yp_s", [N // NCORES, 2], F32, kind="ExternalInput")
    yt_s = nc.dram_tensor("yt_s", [N // NCORES], F32, kind="ExternalInput")
    out_ps = nc.dram_tensor("out_ps", [2 * NW, A], F32, kind="ExternalOutput")
    out_bce = nc.dram_tensor("out_bce", [P, 1], F32, kind="ExternalOutput")
    # scratch for the colsum row->partition rearrange (SBUF->SBUF DMA
    # with partition scatter is broken on HW; DRAM roundtrip works)
    cs_scr = nc.dram_tensor("cs_scr", [NW * NBLK], F32, kind="ExternalOutput")

    # psum bank split points for the [*, A] accumulator (512 f32 / bank)
    banks = [(j, min(j + 512, A)) for j in range(0, A, 512)]
    banks_step = [(b0, min(b1, real_end)) for b0, b1 in banks
                  if b0 < real_end]

    with tile.TileContext(nc) as tc:
        with (
            tc.tile_pool(name="const", bufs=1) as cpool,
            tc.tile_pool(name="work", bufs=2) as wpool,
            tc.tile_pool(name="ind", bufs=2) as ipool,
            tc.tile_pool(name="psum", bufs=1, space=bass.MemorySpace.PSUM) as ppool,
            tc.tile_pool(name="psum_pro", bufs=2,
                         space=bass.MemorySpace.PSUM) as propool,
        ):
            # ---------------- band "a" row: sigmoid + broadcast -------------
            zrow = cpool.tile([1, BAND], F32)
            nc.sync.dma_start(zrow[:], z_band[:].rearrange("(o f) -> o f", o=1))
            srow = cpool.tile([1, BAND], F32)
            nc.scalar.activation(srow[:], zrow[:], AF.Sigmoid)

            ones1 = cpool.tile([1, P], F32)
            nc.vector.memset(ones1[:], 1.0)
            a_band = cpool.tile([P, BAND], F32)
            for j in range(0, BAND, 512):
                w = min(512, BAND - j)
                pb = propool.tile([P, 512], F32, name="pbc", tag="pbc")
                nc.tensor.matmul(pb[:, 0:w], ones1[:], srow[0:1, j:j + w],
                                 start=True, stop=True)
                nc.vector.tensor_copy(a_band[:, j:j + w], pb[:, 0:w])

            # ---------------- b side: pos pred, weights ---------------------
            zbb = wpool.tile([P, 2 * NBLK], F32)
            nc.gpsimd.dma_start(
                zbb[:], yp_b[:].rearrange("(f p) c -> p f c", p=P)
            )
            zbb_v = zbb[:].rearrange("p (f c) -> p c f", c=2)
            mbt = wpool.tile([P, NBLK], F32)
            nc.gpsimd.dma_start(mbt[:], mb[:].rearrange("(f p) -> p f", p=P))
            s_sb = cpool.tile([NBLK, A], BF16)
            nc.gpsimd.dma_start(s_sb[:], s_mat[:])

            zb = wpool.tile([P, NBLK], F32)
            nc.vector.tensor_sub(zb[:], zbb_v[:, 1, :], zbb_v[:, 0, :])
            bm = cpool.tile([P, NBLK], F32)
            nc.scalar.activation(bm[:], zb[:], AF.Sigmoid)
            # compare threshold b - gamma (gamma folded out of the a side)
            bmg = cpool.tile([P, NBLK], F32)
            nc.vector.tensor_scalar(bmg[:], bm[:], -GAMMA, None, op0=OP.add)
            # weight mask: 1 on real b, 0 on pads
            w0 = cpool.tile([P, NBLK], F32)
            nc.vector.tensor_scalar(w0[:], mbt[:], -1.0, 1.0,
                                    op0=OP.mult, op1=OP.add)

            # bf16 weights [w0 | -2bh | -2bm | -2bl | b2h | b2m | b2l];
            # the -2 scale (exact in bf16) pre-folds the cross term of
            # (a-b)^2.  b, b^2 masked to 0 on pad lanes before splitting.
            bz = cpool.tile([P, NBLK], F32)
            nc.vector.tensor_mul(bz[:], bm[:], w0[:])
            b2z = cpool.tile([P, NBLK], F32)
            nc.vector.tensor_mul(b2z[:], bz[:], bm[:])
            rhs_all = cpool.tile([P, NW * NBLK], BF16)
            nc.vector.tensor_copy(rhs_all[:, 0:NBLK], w0[:])
            for base, src, scl in ((1, bz, -2.0), (4, b2z, 1.0)):
                resid = src
                for k in range(3):
                    dst = rhs_all[:, (base + k) * NBLK:(base + k + 1) * NBLK]
                    nc.vector.tensor_scalar(dst, resid[:], scl, None,
                                            op0=OP.mult)
                    if k < 2:
                        back = wpool.tile([P, NBLK], F32, name=f"back{base}{k}",
                                          tag="back")
                        nc.scalar.mul(back[:], dst, 1.0 / scl)
                        nresid = wpool.tile([P, NBLK], F32, name=f"res{base}{k}",
                                            tag="resid")
                        nc.vector.tensor_sub(nresid[:], resid[:], back[:])
                        resid = nresid
            rhs_v = rhs_all[:].rearrange("p (k c) -> p c k", k=NW)

            # per-block weight column-sums -> step stationary [NBLK, 2*NW]
            # (hi/lo bf16 split of the f32 sums for ~18-bit precision)
            ones_c = cpool.tile([P, 1], BF16)
            nc.vector.memset(ones_c[:], 1.0)
            pcs = propool.tile([1, NW * NBLK], F32, name="pcs", tag="pcs")
            nc.tensor.matmul(pcs[:], ones_c[:], rhs_all[:],
                             start=True, stop=True)
            cs_row = wpool.tile([1, NW * NBLK], F32)
            nc.vector.tensor_copy(cs_row[:], pcs[:])
            cs64 = wpool.tile([NBLK, NW], F32)
            nc.sync.dma_start(cs_scr[:].rearrange("(o f) -> o f", o=1), cs_row[:])
            nc.sync.dma_start(
                cs64[:], cs_scr[:].rearrange("(r c) -> c r", c=NBLK)
            )
            cs2 = cpool.tile([NBLK, 2 * NW], BF16)
            nc.vector.tensor_copy(cs2[:, 0:NW], cs64[:])
            cs_back = wpool.tile([NBLK, NW], F32)
            nc.vector.tensor_copy(cs_back[:], cs2[:, 0:NW])
            cs_res = wpool.tile([NBLK, NW], F32)
            nc.vector.tensor_sub(cs_res[:], cs64[:], cs_back[:])
            nc.vector.tensor_copy(cs2[:, NW:2 * NW], cs_res[:])

            # expanded compare threshold bmx[p, base_g + k*Wg + j] = b - gamma
            bmx = cpool.tile([P, BAND], F32)
            for g, (Wg, blks) in enumerate(classes):
                for k, c in enumerate(blks):
                    b0 = cbase[g] + k * Wg
                    nc.vector.tensor_scalar(
                        bmx[:, b0:b0 + Wg],
                        bmg[:, c:c + 1].broadcast_to((P, Wg)),
                        1.0, None, op0=OP.mult,
                    )

            # ---------------- main loop ------------------------------------
            psum = ppool.tile([2 * NW, A], F32)
            mm_blocks = [c for c in range(NBLK) if wid[c] > 0]
            last_c = mm_blocks[-1] if mm_blocks else None

            for rep in range(repeat):
                inds = {}
                for g, (Wg, blks) in enumerate(classes):
                    nb = len(blks)
                    ind = ipool.tile([P, nb * Wg], BF16,
                                     name=f"ind{g}", tag="ind")
                    c0, c1 = cbase[g], cbase[g] + nb * Wg
                    nc.vector.tensor_tensor(
                        ind[:], a_band[:, c0:c1], bmx[:, c0:c1], op=OP.is_gt,
                    )
                    inds[g] = ind
                # step matmuls: all-1 suffix contribution, zero rows 0:14
                for b0, b1 in banks_step:
                    nc.tensor.matmul(
                        psum[0:2 * NW, b0:b1], cs2[:], s_sb[:, b0:b1],
                        start=True, stop=False, skip_group_check=True,
                    )
                # band matmuls (stream only the exact width wid[c])
                for c in mm_blocks:
                    o, w = off[c], wid[c]
                    g, i0 = slot[c]
                    i0 -= cbase[g]
                    segs = []
                    for b0, b1 in banks:
                        s0, s1 = max(o, b0), min(o + w, b1)
                        if s0 < s1:
                            segs.append((s0, s1))
                    for s0, s1 in segs:
                        sp = (rep == repeat - 1 and c == last_c
                              and (s0, s1) == segs[-1])
                        nc.tensor.matmul(
                            psum[0:NW, s0:s1],
                            rhs_v[:, c, :],
                            inds[g][:, i0 + (s0 - o):i0 + (s1 - o)],
                            start=False, stop=sp, skip_group_check=True,
                        )

            # ------------- bce over this core's N/8 rows (host sums) -------
            FC_ = N // NCORES // P
            zff = wpool.tile([P, 2 * FC_], F32)
            nc.gpsimd.dma_start(
                zff[:], yp_s[:].rearrange("(f p) c -> p f c", p=P)
            )
            zff_v = zff[:].rearrange("p (f c) -> p c f", c=2)
            ytb = wpool.tile([P, FC_], F32)
            nc.gpsimd.dma_start(ytb[:], yt_s[:].rearrange("(f p) -> p f", p=P))
            zf = wpool.tile([P, FC_], F32)
            nc.vector.tensor_sub(zf[:], zff_v[:, 1, :], zff_v[:, 0, :])
            pf = wpool.tile([P, FC_], F32)
            nc.scalar.activation(pf[:], zf[:], AF.Sigmoid)
            lp = wpool.tile([P, FC_], F32)
            nc.scalar.activation(lp[:], pf[:], AF.Ln)
            nc.vector.tensor_scalar(lp[:], lp[:], -100.0, None, op0=OP.max)
            q1 = wpool.tile([P, FC_], F32)
            nc.vector.tensor_scalar(q1[:], pf[:], -1.0, 1.0,
                                    op0=OP.mult, op1=OP.add)
            lq = wpool.tile([P, FC_], F32)
            nc.scalar.activation(lq[:], q1[:], AF.Ln)
            nc.vector.tensor_scalar(lq[:], lq[:], -100.0, None, op0=OP.max)
            dd = wpool.tile([P, FC_], F32)
            nc.vector.tensor_sub(dd[:], lp[:], lq[:])
            mmt = wpool.tile([P, FC_], F32)
            nc.vector.tensor_mul(mmt[:], dd[:], ytb[:])
            term = wpool.tile([P, FC_], F32)
            nc.vector.tensor_add(term[:], mmt[:], lq[:])
            bce_sb = wpool.tile([P, 1], F32)
            nc.vector.tensor_reduce(
                bce_sb[:], term[:], axis=mybir.AxisListType.X, op=OP.add
            )

            # ---------------- outputs (cols >= real_end stay zero) ---------
            re_ = real_end
            h_ = re_ // 2
            ps_sb0 = wpool.tile([2 * NW, A], F32)
            nc.vector.tensor_copy(ps_sb0[:, 0:h_], psum[0:2 * NW, 0:h_])
            nc.scalar.copy(ps_sb0[:, h_:re_], psum[0:2 * NW, h_:re_])
            nc.sync.dma_start(out_ps[:, 0:re_], ps_sb0[:, 0:re_])
            nc.sync.dma_start(out_bce[:], bce_sb[:])

    nc.compile()
    return nc


_NC_CACHE = {}


def _get_nc(A, B, classes, off, wid, real_end):
    key = (A, B, tuple((w, tuple(b)) for w, b in classes),
           tuple(off), tuple(wid), real_end)
    if key not in _NC_CACHE:
        _NC_CACHE[key] = build_nc(A, B, classes, off, wid, real_end)
    return _NC_CACHE[key]


def _pad_up(n, m):
    return max(m, ((n + m - 1) // m) * m)


def make_plan(y_pred, y_true):
    """Host-side compaction + sort + band classification."""
    yp = np.ascontiguousarray(np.asarray(y_pred, dtype=np.float32))
    yt64 = np.asarray(y_true).astype(np.int64)
    yt = yt64.astype(np.float32)

    z = (yp[:, 1].astype(np.float64) - yp[:, 0].astype(np.float64))
    sig = 1.0 / (1.0 + np.exp(-z))

    neg_idx = np.where(yt64 == 1)[0]
    pos_idx = np.where(yt64 == 0)[0]
    neg_idx = neg_idx[np.argsort(z[neg_idx], kind="stable")]
    pos_idx = pos_idx[np.argsort(z[pos_idx], kind="stable")]
    nn, npos = len(neg_idx), len(pos_idx)

    B = _pad_up(npos, P)
    yp_b = np.zeros((B, 2), np.float32)
    yp_b[:npos] = yp[pos_idx]
    mb_v = np.ones((B,), np.float32)
    mb_v[:npos] = 0.0

    A = _pad_up((nn + NCORES - 1) // NCORES, P)
    NBLK = B // P

    # f64 a bounds per element, over the global rank window shared by
    # all cores (element i of core k holds sorted rank 8i+k); pads
    # (a = A_PAD, sorting last) appear on some core iff 8(i+1) > nn.
    av = np.full((NCORES * A,), A_PAD, np.float64)
    av[:nn] = sig[neg_idx] + GAMMA
    awin = av.reshape(A, NCORES)
    a_lo = awin.min(axis=1) - EPS
    a_hi = awin.max(axis=1) + EPS

    # f64 b bounds per block over REAL b only (pad weights are zeroed)
    bv = sig[pos_idx]
    b_lo = np.empty(NBLK)
    b_hi = np.empty(NBLK)
    skip = np.zeros(NBLK, bool)
    for c in range(NBLK):
        blk = bv[c * P:min((c + 1) * P, npos)]
        if len(blk) == 0:
            skip[c] = True
            b_lo[c] = b_hi[c] = 2.0
            continue
        b_lo[c] = blk.min() - EPS
        b_hi[c] = blk.max() + EPS

    # band [lo_c, hi_c): outside it the indicator is certainly 0 / 1
    lo = np.searchsorted(a_hi, b_lo, side="right")
    hi = np.searchsorted(a_lo, b_hi, side="left")
    off = lo.astype(np.int64)
    wid = np.maximum(hi - lo, 0).astype(np.int64)
    wid[skip] = 0
    off[skip] = 0
    s_end = off + wid

    # group banded blocks into width classes: each class is one DVE
    # instruction (~151 cycle overhead) over |blks| * Wg padded cols --
    # DP over sorted widths minimizes overhead + padding.
    banded = [c for c in range(NBLK) if wid[c] > 0]
    banded.sort(key=lambda c: wid[c])
    ws = [int(wid[c]) for c in banded]
    nb = len(ws)
    classes = []
    if nb:
        INSTR = 151
        cost = [0.0] * (nb + 1)
        prev = [0] * (nb + 1)
        for i in range(1, nb + 1):
            cost[i] = np.inf
            for j in range(i):
                # blocks j..i-1 in one class of width ws[i-1]
                cc = cost[j] + INSTR + (i - j) * ws[i - 1]
                if cc < cost[i]:
                    cost[i] = cc
                    prev[i] = j
        cuts = []
        i = nb
        while i > 0:
            cuts.append((prev[i], i))
            i = prev[i]
        for j, i in reversed(cuts):
            classes.append((ws[i - 1], banded[j:i]))

    # suffix matrix: S[c, i] = 1 iff i >= s_end[c] (0 row if skip)
    import ml_dtypes
    s_np = (np.arange(A)[None, :] >= s_end[:, None]) & ~skip[:, None]
    s_np = s_np.astype(ml_dtypes.bfloat16)
    real_end = int(-(-nn // NCORES))  # cols beyond this are pad 'a'

    # band layout: class-concatenated, slot k of class g = Wg cols
    band_gi = []  # global a-index per band col
    for Wg, blks in classes:
        for c in blks:
            band_gi.extend(range(off[c], off[c] + Wg))
    band_gi = np.asarray(band_gi, np.int64)
    BAND = len(band_gi)

    maps = []
    a_host = np.empty((NCORES, A), np.float64)
    m_host = np.zeros((NCORES, A), np.float64)
    for c in range(NCORES):
        sh = neg_idx[c::NCORES]
        a_host[c] = A_PAD
        a_host[c, :len(sh)] = sig[sh] + GAMMA
        m_host[c, :len(sh)] = 1.0
        gi = band_gi * NCORES + c
        zb_v = np.where(gi < nn, z[neg_idx[np.minimum(gi, nn - 1)]],
                        30.0).astype(np.float32)
        sl = slice(c * (N // NCORES), (c + 1) * (N // NCORES))
        maps.append({
            "z_band": np.ascontiguousarray(zb_v),
            "yp_b": yp_b, "mb": mb_v, "s_mat": np.ascontiguousarray(s_np),
            "yp_s": np.ascontiguousarray(yp[sl]),
            "yt_s": np.ascontiguousarray(yt[sl]),
        })
    return dict(A=A, B=B, classes=classes, off=off, wid=wid, maps=maps,
                a_host=a_host, m_host=m_host, BAND=BAND, real_end=real_end)


def combine(plan, res):
    """Apply masked a-polynomials to the psum partials (host, f64)."""
    s2 = 0.0
    cnt = 0.0
    bces = []
    for c in range(NCORES):
        o = res.results[c]
        # cols >= real_end are never written on device (pad 'a' slots,
        # masked by m) -- scrub in case the stale psum holds non-finite
        ps = np.nan_to_num(o["out_ps"].astype(np.float64),
                           nan=0.0, posinf=0.0, neginf=0.0)
        a = plan["a_host"][c]
        m = plan["m_host"][c]
        # rows: [0:7] bands + step hi, [7:14] step lo
        K = ps[0] + ps[NW]
        S1 = ps[1:4].sum(0) + ps[NW + 1:NW + 4].sum(0)
        S2c = ps[4:NW].sum(0) + ps[NW + 4:2 * NW].sum(0)
        s2 += float((m * (a * a * K + a * S1 + S2c)).sum())
        cnt += float((m * K).sum())
        bces.append(o["out_bce"].astype(np.float64).sum())
    count = round(cnt)
    bce = -np.sum(bces) / N
    auc = s2 / max(count, 1)
    loss = ALPHA * bce + (1.0 - ALPHA) * auc if count > 0 else bce
    return np.array(loss, dtype=np.float32)


def run_hw(y_pred, y_true, trace=False, **kw):
    plan = make_plan(y_pred, y_true)
    nc = _get_nc(plan["A"], plan["B"], plan["classes"], plan["off"],
                 plan["wid"], plan["real_end"])
    res = run_bass_kernel_spmd(nc, plan["maps"], list(range(NCORES)),
                               trace=trace, **kw)
    return combine(plan, res), res


def kernel(y_pred, y_true):
    loss, _ = run_hw(y_pred, y_true)
    return loss


if __name__ == "__main__":
    # local CoreSim self-test on each core's inputs
    from concourse.bass_interp import CoreSim

    rng = np.random.default_rng(0)
    y_pred = rng.standard_normal((N, 2), dtype=np.float32)
    y_true = rng.integers(0, 2, size=(N,)).astype(np.int64)

    plan = make_plan(y_pred, y_true)
    A, B = plan["A"], plan["B"]
    print(f"A={A} B={B} BAND={plan['BAND']} "
          f"classes={[(w, len(b)) for w, b in plan['classes']]}")
    nc = build_nc(A, B, plan["classes"], plan["off"], plan["wid"],
                  plan["real_end"])

    pred = 1.0 / (1.0 + np.exp(-(y_pred[:, 1] - y_pred[:, 0]).astype(np.float64)))
    yt = y_true.astype(np.float64)
    lp = np.maximum(np.log(pred), -100)
    lq = np.maximum(np.log1p(-pred), -100)
    bce_all = yt * lp + (1 - yt) * lq
    neg_idx = np.where(y_true == 1)[0]
    pos_idx = np.where(y_true == 0)[0]
    zi = y_pred[:, 1].astype(np.float64) - y_pred[:, 0].astype(np.float64)
    order = neg_idx[np.argsort(zi[neg_idx], kind="stable")]
    b = pred[pos_idx]

    class FakeRes:
        results = []

    for core in range(2):
        sim = CoreSim(nc)
        for k, v in plan["maps"][core].items():
            sim.tensor(k)[:] = v
        sim.simulate(check_with_hw=False)
        o = {"out_ps": np.array(sim.tensor("out_ps")),
             "out_bce": np.array(sim.tensor("out_bce"))}
        FakeRes.results.append(o)

        a = pred[order[core::NCORES]] + GAMMA
        d = a[:, None] - b[None, :]
        msk = d > 0
        s2_ref = (np.where(msk, d, 0.0) ** 2).sum()
        k_ref = msk.sum()
        bce_ref = bce_all[core * (N // NCORES):(core + 1) * (N // NCORES)].sum()

        ps = np.nan_to_num(o["out_ps"].astype(np.float64),
                           nan=0.0, posinf=0.0, neginf=0.0)
        ah = plan["a_host"][core]
        mh = plan["m_host"][core]
        K = ps[0] + ps[NW]
        S1 = ps[1:4].sum(0) + ps[NW + 1:NW + 4].sum(0)
        S2c = ps[4:NW].sum(0) + ps[NW + 4:2 * NW].sum(0)
        s2_dev = float((mh * (ah * ah * K + ah * S1 + S2c)).sum())
        k_dev = float((mh * K).sum())
        bce_dev = o["out_bce"].astype(np.float64).sum()
        print(f"core{core}: S2 relerr={abs(s2_dev-s2_ref)/abs(s2_ref):.3e} "
              f"K err={k_dev-k_ref:.1f} "
              f"BCE relerr={abs(bce_dev-bce_ref)/abs(bce_ref):.3e}")
